# revision 35
# baseline (speedup 1.0000x reference)
"""CascadeNNBN Trainium2 kernel.

8-way data-parallel over the batch dim. Each core holds a 2048-row shard
of the batch with features kept TRANSPOSED in SBUF (features on
partitions, batch on the free axis), so every cascade matmul contracts
over the partition dim with no on-device transposes:

    h_i^T [256, 2048] = W_i @ feats^T   (lhsT = W_i^T, host-pretransposed)

BatchNorm batch statistics are raw per-core (sum, sum-of-squares) pairs,
exchanged with one tiny AllGather per stage and summed locally.  Only
the last two k-tiles of each stage's contraction depend on the exchanged
stats (BN is a per-feature affine transform), so each collective's
latency is hidden behind AR-independent matmuls: the next stage's
ungated k-tiles plus output-matmul tiles spread across the windows.

The first collective pays a large one-time rendezvous barrier
(~35-55us of CC-core boot + cross-core skew), so the x-block partial
sums of stages 2-7 are pre-accumulated ("parked" in SBUF, re-injected
exactly via an identity matmul) to give the first windows enough
independent matmul work.  DMA queue assignment is deliberate: the sync
ring carries the per-stage stats sends (and must drain its startup
loads by ~25us), the scalar ring carries the bulk weights, and the
gpsimd queue stays empty so the collective triggers issue promptly.

Matmuls run in bf16 (fp32 PSUM accumulation); statistics, normalization
coefficients and the final output are fp32.
"""

import sys

if "/opt/trn_rl_repo" not in sys.path:
    sys.path.insert(0, "/opt/trn_rl_repo")

import numpy as np
from ml_dtypes import bfloat16

import concourse.bass as bass  # noqa: F401  (import keeps bass registered)
import concourse.mybir as mybir
import concourse.tile as tile
from concourse import bacc
from concourse.bass_utils import run_bass_kernel_spmd


N_CORES = 8
B = 16384
BSH = B // N_CORES          # 2048 batch rows per core
DIN = 512
K = 8                       # cascade stages
WS = 256                    # neurons per stage
DOUT = 128
EPS = 1e-5
P = 128
NB = BSH // 512             # batch chunks of 512 (PSUM bank free dim)
KO = [(DIN + WS * i) // P for i in range(K)]   # k-tiles per stage: 4,6,...,18
T_TOT = (DIN + WS * K) // P                    # 20 F tiles
PARK = (2, 3, 4, 5, 6, 7)   # stages whose x-part is parked early (2-5 in
                            # window 0, 6 in window 1, 7 in window 2)

_NC_CACHE = {}

# test-harness knobs (ignored in normal use): when TRACE_DIR is set the
# device run is profiled and kernel() stores the BassKernelResults here.
TRACE_DIR = None
LAST_RESULTS = None

BF = mybir.dt.bfloat16
F32 = mybir.dt.float32


def _build_nc():
    nc = bacc.Bacc("TRN2", target_bir_lowering=False, debug=False,
                   num_devices=N_CORES)

    xt_d = nc.dram_tensor("xt", [P, DIN // P, BSH], BF, kind="ExternalInput")
    w_d = [
        nc.dram_tensor(f"w{i}", [P, KO[i], WS], BF, kind="ExternalInput")
        for i in range(K)
    ]
    wo_d = nc.dram_tensor("wo", [P, T_TOT, DOUT], BF, kind="ExternalInput")
    bv_d = nc.dram_tensor("bv", [P, K, 2], F32, kind="ExternalInput")
    gv_d = nc.dram_tensor("gv", [P, K, 2], F32, kind="ExternalInput")
    bev_d = nc.dram_tensor("bev", [P, K, 2], F32, kind="ExternalInput")
    bout_d = nc.dram_tensor("boutv", [P, 1], F32, kind="ExternalInput")
    outT_d = nc.dram_tensor("outT", [P, BSH], F32, kind="ExternalOutput")

    with tile.TileContext(nc) as tc:
        _emit(nc, tc, xt_d, w_d, wo_d, bv_d, gv_d, bev_d, bout_d, outT_d)
    nc.compile()
    return nc


def _emit(nc, tc, xt_d, w_d, wo_d, bv_d, gv_d, bev_d, bout_d, outT_d):
    AF = mybir.ActivationFunctionType
    OP = mybir.AluOpType
    groups = [list(range(N_CORES))]

    with (
        tc.tile_pool(name="big", bufs=1) as big,
        tc.tile_pool(name="hp", bufs=1) as hp,
        tc.tile_pool(name="scrp", bufs=3) as scrp,
        tc.tile_pool(name="small", bufs=2) as small,
        tc.tile_pool(name="ps", bufs=8, space="PSUM") as ps,
        tc.tile_pool(name="dram", bufs=2, space="DRAM") as dram,
    ):
        # ---- persistent SBUF ----
        F = [big.tile([P, BSH], BF, tag=f"F{t}", name=f"F{t}") for t in range(T_TOT)]
        Wsb = [big.tile([P, KO[i], WS], BF, tag=f"W{i}", name=f"W{i}") for i in range(K)]
        WO = big.tile([P, T_TOT, DOUT], BF, tag="WO")
        BV = big.tile([P, K, 2], F32, tag="BV")
        GV = big.tile([P, K, 2], F32, tag="GV")
        BEV = big.tile([P, K, 2], F32, tag="BEV")
        BOUT = big.tile([P, 1], F32, tag="BOUT")
        OUTACC = big.tile([P, BSH], F32, tag="OUTACC")
        # eps * B^2: the variance chain works on B^2-scaled moments
        EPSB2 = big.tile([P, 1], F32, tag="EPSB2")
        nc.vector.memset(EPSB2[:], EPS * float(B) * float(B))
        # gamma * B (amortizes the 1/B de-scaling of the global stats)
        GVB = big.tile([P, K, 2], F32, tag="GVB")
        # x-block partial pre-accumulation targets (parked during window 0)
        HACC = {j: big.tile([P, 2, BSH], BF, tag=f"HACC{j}", name=f"HACC{j}")
                for j in PARK}

        # ---- input DMAs ----
        # Stage-0 weights ride the fast hardware rings ahead of x so the
        # first matmul can start within ~2us; gpsimd (software DMA) only
        # carries the output weights and small vectors, then the per-stage
        # stats sends.  The AllGather read-backs ride sync (hardware),
        # whose ring is clear after the startup loads.
        nc.sync.dma_start(Wsb[0][:, 0, :], w_d[0][:, 0, :])
        nc.sync.dma_start(Wsb[0][:, 1, :], w_d[0][:, 1, :])
        nc.scalar.dma_start(Wsb[0][:, 2, :], w_d[0][:, 2, :])
        nc.scalar.dma_start(Wsb[0][:, 3, :], w_d[0][:, 3, :])
        # small vectors on the hardware rings (software-DGE issues from
        # gpsimd cost ~3.5us each)
        nc.sync.dma_start(BV[:], bv_d[:, :, :])
        nc.sync.dma_start(BOUT[:], bout_d[:, :])
        nc.scalar.dma_start(GV[:], gv_d[:, :, :])
        nc.scalar.dma_start(BEV[:], bev_d[:, :, :])
        # x spread round-robin over the sync/scalar rings in consumption
        # order
        xengs = [nc.sync, nc.scalar]
        idx = 0
        for t in range(DIN // P):
            for bb in range(NB):
                xengs[idx % 2].dma_start(F[t][:, bb * 512:(bb + 1) * 512],
                                         xt_d[:, t, bb * 512:(bb + 1) * 512])
                idx += 1
        # The hardware rings run at ~65GB/s, and the sync ring must drain
        # by ~25us because it carries the per-stage stats sends, while the
        # gpsimd queue must stay empty: it issues the collective triggers,
        # and software-DGE descriptor generation would delay them.  So W1
        # alone follows x on sync, and everything else rides scalar: the
        # parked stages' x-part k-tiles first (all that window-0 parking
        # needs), the bn-part remainders and W6/W7 after.
        nc.sync.dma_start(Wsb[1][:], w_d[1][:, :, :])
        for j in (2, 3, 4, 5):
            nc.scalar.dma_start(Wsb[j][:, 0:4, :], w_d[j][:, 0:4, :])
        nc.scalar.dma_start(WO[:], wo_d[:, :, :])
        for j in (2, 3, 4, 5):
            nc.scalar.dma_start(Wsb[j][:, 4:KO[j], :], w_d[j][:, 4:KO[j], :])
        nc.scalar.dma_start(Wsb[6][:], w_d[6][:, :, :])
        nc.scalar.dma_start(Wsb[7][:], w_d[7][:, :, :])
        nc.vector.tensor_scalar_mul(GVB[:], GV[:, :, :], float(B))

        PREACC = {j: 4 for j in PARK}  # parked k-tiles (the x block)

        def mm(pt, lhsT, k, bb, start, stop):
            nc.tensor.matmul(
                pt[:, :], lhsT, F[k][:, bb * 512:(bb + 1) * 512],
                start=start, stop=stop)

        def alloc_group():
            return [
                [ps.tile([P, 512], F32, tag="pt", name="pt") for _ in range(NB)]
                for _ in range(2)
            ]

        def park_group(j):
            """Pre-accumulate stage j's x-part into HACC[j] (SBUF, bf16).

            The PSUM->SBUF drains are split across ScalarE (n=0) and
            VectorE (n=1): a single engine's serial drain (~5.5us) would
            stall the next group's PSUM-bank allocation."""
            pa = alloc_group()
            for n in range(2):
                for k in range(4):
                    lhsT = Wsb[j][:, k, n * P:(n + 1) * P]
                    for bb in range(NB):
                        mm(pa[n][bb], lhsT, k, bb, k == 0, k == 3)
            for bb in range(NB):
                sl = slice(bb * 512, (bb + 1) * 512)
                nc.scalar.activation(HACC[j][:, 0, sl], pa[0][bb][:, :],
                                     AF.Copy)
                nc.vector.tensor_copy(HACC[j][:, 1, sl], pa[1][bb][:, :])

        def real_early(j, psums):
            """Ungated part of stage j's contraction.

            For parked stages the SBUF partial sum is added into PSUM by
            VectorE right after each n-half's early matmuls: cheaper than
            an identity-matmul re-injection (which costs PE cycles), and
            the n=0 adds run while the PE streams the n=1 half."""
            pre = PREACC.get(j, 0)
            for n in range(2):
                for k in range(pre, KO[j] - 2):
                    lhsT = Wsb[j][:, k, n * P:(n + 1) * P]
                    for bb in range(NB):
                        mm(psums[n][bb], lhsT, k, bb, k == pre, False)
                if pre:
                    for bb in range(NB):
                        nc.vector.tensor_add(
                            psums[n][bb][:, :], psums[n][bb][:, :],
                            HACC[j][:, n, bb * 512:(bb + 1) * 512])

        def real_late(j, psums):
            # chunk-major order: psum (n, bb) groups complete progressively
            # so the relu/stats pipeline starts before the last matmul
            for bb in range(NB):
                for n in range(2):
                    for k in (KO[j] - 2, KO[j] - 1):
                        lhsT = Wsb[j][:, k, n * P:(n + 1) * P]
                        mm(psums[n][bb], lhsT, k, bb, False, k == KO[j] - 1)

        def out_group(ks, first=False):
            pso = [ps.tile([P, 512], F32, tag="pt", name="pt")
                   for _ in range(NB)]
            for k in ks:
                lhsT = WO[:, k, :]
                for bb in range(NB):
                    nc.tensor.matmul(
                        pso[bb][:, :], lhsT,
                        F[k][:, bb * 512:(bb + 1) * 512],
                        start=(k == ks[0]), stop=(k == ks[-1]))
            for bb in range(NB):
                dst = OUTACC[:, bb * 512:(bb + 1) * 512]
                if first:
                    nc.vector.tensor_scalar_add(dst, pso[bb][:, :],
                                                BOUT[:, 0:1])
                else:
                    nc.vector.tensor_add(dst, dst, pso[bb][:, :])

        # per-window AR-independent fill, sized to each collective's
        # exposed latency (window 0 carries the one-time barrier cost)
        WINDOW_FILL = {
            0: [lambda: park_group(2),
                lambda: park_group(3),
                lambda: park_group(4),
                lambda: park_group(5)],
            1: [lambda: out_group((0, 1), True),
                lambda: park_group(6)],
            2: [lambda: out_group((2, 3)),
                lambda: park_group(7)],
            7: [lambda: out_group(range(4, T_TOT - 2))],
        }

        # stage 0: everything available immediately
        psums = alloc_group()
        real_early(0, psums)
        real_late(0, psums)

        for i in range(K):
            # ---- relu + bias: PSUM -> bf16 h in SBUF ----
            # n=0 chunks on ScalarE, n=1 chunks on VectorE so both n-tiles
            # clear PSUM (and feed bn_stats) in parallel.
            hs = [hp.tile([P, BSH], BF, tag=f"h{n}", name=f"h{n}") for n in range(2)]
            sums = small.tile([P, NB], F32, tag="sums")
            sqs = small.tile([P, NB], F32, tag="sqs")
            st = small.tile([P, NB, 6], F32, tag="st")
            mv1 = small.tile([P, 2], F32, tag="mv1")
            # payload lanes: (S, B*Q, S*gamma) per n-half -- all additive
            # across cores, chosen so the post-AG coefficient chain is short
            arin = small.tile([P, 2, 3], F32, tag="arin")
            # stats payload = raw (sum, sum-of-squares) pairs, additive
            # across cores. n=0 on ScalarE (relu/square with accum_out),
            # n=1 on VectorE (relu + bn_stats), in psum-completion order so
            # both pipelines drain right behind the last matmul.
            for bb in range(NB):
                c0 = hs[0][:, bb * 512:(bb + 1) * 512]
                c1 = hs[1][:, bb * 512:(bb + 1) * 512]
                nc.scalar.activation(
                    c0, psums[0][bb][:, :], AF.Relu,
                    bias=BV[:, i, 0:1], scale=1.0,
                    accum_out=sums[:, bb:bb + 1],
                )
                scr = scrp.tile([P, 512], BF, tag="scr", name="scr")
                nc.scalar.activation(
                    scr[:, :], c0, AF.Square,
                    accum_out=sqs[:, bb:bb + 1])
                nc.vector.tensor_scalar(
                    c1, psums[1][bb][:, :], BV[:, i, 1:2], 0.0,
                    op0=OP.add, op1=OP.max,
                )
                nc.vector.bn_stats(st[:, bb, :], c1)
            nc.vector.bn_aggr(mv1[:], st[:, :, :])
            # n=1: (mean, var) -> raw (S, Q);  n=0: reduce the chunk sums
            nc.vector.tensor_scalar(
                arin[:, 1, 1:2], mv1[:, 0:1], mv1[:, 0:1], mv1[:, 1:2],
                op0=OP.mult, op1=OP.add)
            nc.vector.tensor_scalar_mul(arin[:, 1, 1:2], arin[:, 1, 1:2],
                                        float(BSH))
            nc.vector.tensor_scalar_mul(arin[:, 1, 0:1], mv1[:, 0:1],
                                        float(BSH))
            nc.vector.tensor_reduce(
                arin[:, 0, 0:1], sums[:, :], axis=mybir.AxisListType.X,
                op=OP.add)
            nc.vector.tensor_reduce(
                arin[:, 0, 1:2], sqs[:, :], axis=mybir.AxisListType.X,
                op=OP.add)
            # Q -> B*Q (both halves) and the S*gamma lane
            nc.vector.tensor_scalar_mul(arin[:, :, 1:2], arin[:, :, 1:2],
                                        float(B))
            nc.vector.tensor_mul(arin[:, :, 2], arin[:, :, 0], GV[:, i, :])

            # ---- cross-core exchange of the stats (3KB): AllGather has a
            # ~2x lower latency floor than AllReduce; the 8-way sum is done
            # locally on VectorE.  The send rides the gpsimd ring (idle);
            # the read-back is split by partition halves across the sync
            # and scalar rings to halve its latency. ----
            ccin = dram.tile([P, 2, 3], F32, tag="ccin")
            ccout = dram.tile([N_CORES, P, 2, 3], F32, tag="ccout",
                              addr_space="Shared")
            nc.sync.dma_start(ccin[:], arin[:])
            nc.gpsimd.collective_compute(
                "AllGather", OP.bypass, replica_groups=groups,
                ins=[ccin.opt()], outs=[ccout.opt()],
            )
            ag = small.tile([P, N_CORES, 2, 3], F32, tag="ag")
            HP = P // 2
            nc.sync.dma_start(ag[0:HP], ccout[:, 0:HP, :, :].rearrange(
                "r p a b -> p r a b"))

            # ---- overlap window: AR-independent matmul fill ----
            for thunk in WINDOW_FILL.get(i, []):
                thunk()
            if i < K - 1:
                psums = alloc_group()
                real_early(i + 1, psums)

            # second half of the read-back: emitted after the fill so the
            # scalar queue's park copies run during the collective
            nc.scalar.dma_start(ag[HP:P], ccout[:, HP:P, :, :].rearrange(
                "r p a b -> p r a b"))
            ared = small.tile([P, 2, 3], F32, tag="ared")
            nc.vector.tensor_reduce(
                ared[:, :, :], ag[:, :, :, :].rearrange("p r a b -> p a b r"),
                axis=mybir.AxisListType.X, op=OP.add)

            # ---- BN affine coefficients from the (S, B*Q, S*gamma)
            # global sums:  B^2*var = (B*Q) - S^2 ;
            # a = gamma/std = (B*gamma) * r ;  negc = a*mu - beta =
            # (S*gamma) * r - beta   with r = 1/sqrt(B^2*var + B^2*eps) ----
            T1 = small.tile([P, 2], F32, tag="T1")
            T3 = small.tile([P, 2], F32, tag="T3")
            rstd = small.tile([P, 2], F32, tag="rstd")
            a_ = small.tile([P, 2], F32, tag="a_")
            negc = small.tile([P, 2], F32, tag="negc")
            nc.vector.tensor_mul(T1[:], ared[:, :, 0], ared[:, :, 0])
            nc.vector.tensor_sub(T3[:], ared[:, :, 1], T1[:])
            nc.scalar.activation(rstd[:], T3[:], AF.Sqrt,
                                 bias=EPSB2[:, 0:1], scale=1.0)
            nc.vector.reciprocal(rstd[:], rstd[:])
            nc.vector.tensor_mul(a_[:], GVB[:, i, :], rstd[:])
            nc.vector.tensor_mul(negc[:], ared[:, :, 2], rstd[:])
            nc.vector.tensor_sub(negc[:], negc[:], BEV[:, i, :])

            # ---- normalize into the F blocks (bf16), chunk-major (both
            # n-tiles of chunk q before chunk q+1) to match the gated
            # matmuls' chunk-major consumption order ----
            for q in range(NB):
                sl = slice(q * 512, (q + 1) * 512)
                for n in range(2):
                    nc.vector.tensor_scalar(
                        F[DIN // P + 2 * i + n][:, sl], hs[n][:, sl],
                        a_[:, n:n + 1], negc[:, n:n + 1],
                        op0=OP.mult, op1=OP.subtract,
                    )

            # ---- gated (late) matmuls of the next stage ----
            if i < K - 1:
                real_late(i + 1, psums)

        # ---- epilogue: last two output k-tiles, chunk-pipelined with the
        # final store spread across four DMA rings ----
        pso = [ps.tile([P, 512], F32, tag="pt", name="pt") for _ in range(NB)]
        oengs = [nc.sync, nc.scalar]
        for bb in range(NB):
            for k in (T_TOT - 2, T_TOT - 1):
                nc.tensor.matmul(
                    pso[bb][:, :], WO[:, k, :],
                    F[k][:, bb * 512:(bb + 1) * 512],
                    start=(k == T_TOT - 2), stop=(k == T_TOT - 1))
            dst = OUTACC[:, bb * 512:(bb + 1) * 512]
            nc.vector.tensor_add(dst, dst, pso[bb][:, :])
            # store in 256-wide half-chunks alternating rings for overlap
            for h in range(2):
                sl = slice(bb * 512 + h * 256, bb * 512 + (h + 1) * 256)
                oengs[(2 * bb + h) % 2].dma_start(outT_d[:, sl], OUTACC[:, sl])


def _get_nc():
    if "nc" not in _NC_CACHE:
        _NC_CACHE["nc"] = _build_nc()
    return _NC_CACHE["nc"]


def kernel(x, W0, W1, W2, W3, W4, W5, W6, W7, b, gamma, beta, Wout, bout):
    Ws = [W0, W1, W2, W3, W4, W5, W6, W7]
    nc = _get_nc()

    def pack_vec(v):  # [8,256] -> [128, 8, 2]
        return np.ascontiguousarray(
            np.asarray(v, np.float32).reshape(K, 2, P).transpose(2, 0, 1))

    common = {}
    for i, W in enumerate(Ws):
        wt = np.asarray(W, np.float32).T.astype(bfloat16)        # [d_i, 256]
        common[f"w{i}"] = np.ascontiguousarray(
            wt.reshape(KO[i], P, WS).transpose(1, 0, 2))         # [128, ko, 256]
    wot = np.asarray(Wout, np.float32).T.astype(bfloat16)        # [2560, 128]
    common["wo"] = np.ascontiguousarray(
        wot.reshape(T_TOT, P, DOUT).transpose(1, 0, 2))          # [128, 20, 128]
    common["bv"] = pack_vec(b)
    common["gv"] = pack_vec(gamma)
    common["bev"] = pack_vec(beta)
    common["boutv"] = np.ascontiguousarray(
        np.asarray(bout, np.float32).reshape(P, 1))

    in_maps = []
    for c in range(N_CORES):
        xs = np.asarray(x[c * BSH:(c + 1) * BSH], np.float32)    # [2048, 512]
        xt = xs.T.astype(bfloat16)                               # [512, 2048]
        in_maps.append({
            **common,
            "xt": np.ascontiguousarray(
                xt.reshape(DIN // P, P, BSH).transpose(1, 0, 2)),
        })

    kw = {}
    if TRACE_DIR is not None:
        kw = dict(trace=True, tmpdir=TRACE_DIR)
    try:
        res = run_bass_kernel_spmd(nc, in_maps, list(range(N_CORES)), **kw)
    except Exception:
        # transient PJRT INTERNAL errors have been observed; retry once
        res = run_bass_kernel_spmd(nc, in_maps, list(range(N_CORES)), **kw)
    global LAST_RESULTS
    LAST_RESULTS = res
    out = np.empty((B, DOUT), np.float32)
    for c in range(N_CORES):
        out[c * BSH:(c + 1) * BSH] = res.results[c]["outT"].T
    return out


# revision 41
# speedup vs baseline: 1.1138x; 1.1138x over previous
"""CascadeNNBN Trainium2 kernel.

8-way data-parallel over the batch dim. Each core holds a 2048-row shard
of the batch with features kept TRANSPOSED in SBUF (features on
partitions, batch on the free axis), so every cascade matmul contracts
over the partition dim with no on-device transposes:

    h_i^T [256, 2048] = W_i @ feats^T   (lhsT = W_i^T, host-pretransposed)

BatchNorm batch statistics are raw per-core (sum, sum-of-squares) pairs,
exchanged with one tiny AllGather per stage and summed locally.  Only
the last two k-tiles of each stage's contraction depend on the exchanged
stats (BN is a per-feature affine transform), so each collective's
latency is hidden behind AR-independent matmuls: the next stage's
ungated k-tiles plus output-matmul tiles spread across the windows.

The first collective pays a large one-time rendezvous barrier
(~35-55us of CC-core boot + cross-core skew), so the x-block partial
sums of stages 2-7 are pre-accumulated ("parked" in SBUF, re-injected
exactly via an identity matmul) to give the first windows enough
independent matmul work.  DMA queue assignment is deliberate: the sync
ring carries the per-stage stats sends (and must drain its startup
loads by ~25us), the scalar ring carries the bulk weights, and the
gpsimd queue stays empty so the collective triggers issue promptly.

Matmuls run in bf16 (fp32 PSUM accumulation); statistics, normalization
coefficients and the final output are fp32.
"""

import sys

if "/opt/trn_rl_repo" not in sys.path:
    sys.path.insert(0, "/opt/trn_rl_repo")

import numpy as np
from ml_dtypes import bfloat16

import concourse.bass as bass  # noqa: F401  (import keeps bass registered)
import concourse.mybir as mybir
import concourse.tile as tile
from concourse import bacc
from concourse.bass_utils import run_bass_kernel_spmd
from concourse.masks import make_identity


N_CORES = 8
B = 16384
BSH = B // N_CORES          # 2048 batch rows per core
DIN = 512
K = 8                       # cascade stages
WS = 256                    # neurons per stage
DOUT = 128
EPS = 1e-5
P = 128
NB = BSH // 512             # batch chunks of 512 (PSUM bank free dim)
KO = [(DIN + WS * i) // P for i in range(K)]   # k-tiles per stage: 4,6,...,18
T_TOT = (DIN + WS * K) // P                    # 20 F tiles
PARK = (2, 3, 4, 5, 6, 7)   # stages whose x-part is parked early (2-5 in
                            # window 0, 6 in window 1, 7 in window 2)

_NC_CACHE = {}

# test-harness knobs (ignored in normal use): when TRACE_DIR is set the
# device run is profiled and kernel() stores the BassKernelResults here.
TRACE_DIR = None
LAST_RESULTS = None

BF = mybir.dt.bfloat16
F32 = mybir.dt.float32


def _build_nc():
    nc = bacc.Bacc("TRN2", target_bir_lowering=False, debug=False,
                   num_devices=N_CORES)

    xt_d = nc.dram_tensor("xt", [P, DIN // P, BSH], BF, kind="ExternalInput")
    w_d = [
        nc.dram_tensor(f"w{i}", [P, KO[i], WS], BF, kind="ExternalInput")
        for i in range(K)
    ]
    wo_d = nc.dram_tensor("wo", [P, T_TOT, DOUT], BF, kind="ExternalInput")
    bv_d = nc.dram_tensor("bv", [P, K, 2], F32, kind="ExternalInput")
    gv_d = nc.dram_tensor("gv", [P, K, 2], F32, kind="ExternalInput")
    bev_d = nc.dram_tensor("bev", [P, K, 2], F32, kind="ExternalInput")
    bout_d = nc.dram_tensor("boutv", [P, 1], F32, kind="ExternalInput")
    outT_d = nc.dram_tensor("outT", [P, BSH], F32, kind="ExternalOutput")

    with tile.TileContext(nc) as tc:
        _emit(nc, tc, xt_d, w_d, wo_d, bv_d, gv_d, bev_d, bout_d, outT_d)
    nc.compile()
    return nc


def _emit(nc, tc, xt_d, w_d, wo_d, bv_d, gv_d, bev_d, bout_d, outT_d):
    AF = mybir.ActivationFunctionType
    OP = mybir.AluOpType
    groups = [list(range(N_CORES))]

    with (
        tc.tile_pool(name="big", bufs=1) as big,
        tc.tile_pool(name="hp", bufs=1) as hp,
        tc.tile_pool(name="scrp", bufs=3) as scrp,
        tc.tile_pool(name="small", bufs=2) as small,
        tc.tile_pool(name="ps", bufs=8, space="PSUM") as ps,
        tc.tile_pool(name="dram", bufs=2, space="DRAM") as dram,
    ):
        # ---- persistent SBUF ----
        F = [big.tile([P, BSH], BF, tag=f"F{t}", name=f"F{t}") for t in range(T_TOT)]
        Wsb = [big.tile([P, KO[i], WS], BF, tag=f"W{i}", name=f"W{i}") for i in range(K)]
        WO = big.tile([P, T_TOT, DOUT], BF, tag="WO")
        BV = big.tile([P, K, 2], F32, tag="BV")
        GV = big.tile([P, K, 2], F32, tag="GV")
        BEV = big.tile([P, K, 2], F32, tag="BEV")
        BOUT = big.tile([P, 1], F32, tag="BOUT")
        OUTACC = big.tile([P, BSH], F32, tag="OUTACC")
        # eps * B^2: the variance chain works on B^2-scaled moments
        EPSB2 = big.tile([P, 1], F32, tag="EPSB2")
        nc.vector.memset(EPSB2[:], EPS * float(B) * float(B))
        # gamma * B (amortizes the 1/B de-scaling of the global stats)
        GVB = big.tile([P, K, 2], F32, tag="GVB")
        # identity (bf16) for re-injecting SBUF partial sums into PSUM
        IDT = big.tile([P, P], BF, tag="IDT")
        make_identity(nc, IDT[:, :])
        # x-block partial pre-accumulation targets (parked during window 0)
        HACC = {j: big.tile([P, 2, BSH], BF, tag=f"HACC{j}", name=f"HACC{j}")
                for j in PARK}

        # ---- input DMAs ----
        # Stage-0 weights ride the fast hardware rings ahead of x so the
        # first matmul can start within ~2us; gpsimd (software DMA) only
        # carries the output weights and small vectors, then the per-stage
        # stats sends.  The AllGather read-backs ride sync (hardware),
        # whose ring is clear after the startup loads.
        nc.sync.dma_start(Wsb[0][:, 0, :], w_d[0][:, 0, :])
        nc.sync.dma_start(Wsb[0][:, 1, :], w_d[0][:, 1, :])
        nc.scalar.dma_start(Wsb[0][:, 2, :], w_d[0][:, 2, :])
        nc.scalar.dma_start(Wsb[0][:, 3, :], w_d[0][:, 3, :])
        # small vectors on the hardware rings (software-DGE issues from
        # gpsimd cost ~3.5us each)
        nc.sync.dma_start(BV[:], bv_d[:, :, :])
        nc.sync.dma_start(BOUT[:], bout_d[:, :])
        nc.scalar.dma_start(GV[:], gv_d[:, :, :])
        nc.scalar.dma_start(BEV[:], bev_d[:, :, :])
        # x spread round-robin over the sync/scalar rings in consumption
        # order
        xengs = [nc.sync, nc.scalar]
        idx = 0
        for t in range(DIN // P):
            for bb in range(NB):
                xengs[idx % 2].dma_start(F[t][:, bb * 512:(bb + 1) * 512],
                                         xt_d[:, t, bb * 512:(bb + 1) * 512])
                idx += 1
        # The hardware rings run at ~65GB/s, and the sync ring must drain
        # by ~25us because it carries the per-stage stats sends, while the
        # gpsimd queue must stay empty: it issues the collective triggers,
        # and software-DGE descriptor generation would delay them.  So W1
        # alone follows x on sync, and everything else rides scalar: the
        # parked stages' x-part k-tiles first (all that window-0 parking
        # needs), the bn-part remainders and W6/W7 after.
        nc.sync.dma_start(Wsb[1][:], w_d[1][:, :, :])
        for j in (2, 3, 4, 5):
            nc.scalar.dma_start(Wsb[j][:, 0:4, :], w_d[j][:, 0:4, :])
        nc.scalar.dma_start(WO[:], wo_d[:, :, :])
        for j in (2, 3, 4, 5):
            nc.scalar.dma_start(Wsb[j][:, 4:KO[j], :], w_d[j][:, 4:KO[j], :])
        nc.scalar.dma_start(Wsb[6][:], w_d[6][:, :, :])
        nc.scalar.dma_start(Wsb[7][:], w_d[7][:, :, :])
        nc.vector.tensor_scalar_mul(GVB[:], GV[:, :, :], float(B))

        PREACC = {j: 4 for j in PARK}  # parked k-tiles (the x block)

        def mm(pt, lhsT, k, bb, start, stop):
            nc.tensor.matmul(
                pt[:, :], lhsT, F[k][:, bb * 512:(bb + 1) * 512],
                start=start, stop=stop)

        def alloc_group():
            return [
                [ps.tile([P, 512], F32, tag="pt", name="pt") for _ in range(NB)]
                for _ in range(2)
            ]

        def park_group(j):
            """Pre-accumulate stage j's x-part into HACC[j] (SBUF, bf16).

            The PSUM->SBUF drains are split across ScalarE (n=0) and
            VectorE (n=1): a single engine's serial drain (~5.5us) would
            stall the next group's PSUM-bank allocation."""
            pa = alloc_group()
            for n in range(2):
                for k in range(4):
                    lhsT = Wsb[j][:, k, n * P:(n + 1) * P]
                    for bb in range(NB):
                        mm(pa[n][bb], lhsT, k, bb, k == 0, k == 3)
            for bb in range(NB):
                sl = slice(bb * 512, (bb + 1) * 512)
                nc.scalar.activation(HACC[j][:, 0, sl], pa[0][bb][:, :],
                                     AF.Copy)
                nc.vector.tensor_copy(HACC[j][:, 1, sl], pa[1][bb][:, :])

        def real_early(j, psums):
            """Ungated part of stage j's contraction.

            Parked stages re-inject their SBUF partial sum via an exact
            identity matmul.  (A VectorE psum-add was tried instead and
            cost ~30us of wall time: the vector queue is the second
            -critical resource inside the collective windows.)"""
            pre = PREACC.get(j, 0)
            for n in range(2):
                if pre:
                    for bb in range(NB):
                        nc.tensor.matmul(
                            psums[n][bb][:, :], IDT[:, :],
                            HACC[j][:, n, bb * 512:(bb + 1) * 512],
                            start=True, stop=False)
                for k in range(pre, KO[j] - 2):
                    lhsT = Wsb[j][:, k, n * P:(n + 1) * P]
                    for bb in range(NB):
                        mm(psums[n][bb], lhsT, k, bb,
                           pre == 0 and k == 0, False)

        def real_late(j, psums):
            # chunk-major order: psum (n, bb) groups complete progressively
            # so the relu/stats pipeline starts before the last matmul
            for bb in range(NB):
                for n in range(2):
                    for k in (KO[j] - 2, KO[j] - 1):
                        lhsT = Wsb[j][:, k, n * P:(n + 1) * P]
                        mm(psums[n][bb], lhsT, k, bb, False, k == KO[j] - 1)

        def out_group(ks, first=False):
            pso = [ps.tile([P, 512], F32, tag="pt", name="pt")
                   for _ in range(NB)]
            for k in ks:
                lhsT = WO[:, k, :]
                for bb in range(NB):
                    nc.tensor.matmul(
                        pso[bb][:, :], lhsT,
                        F[k][:, bb * 512:(bb + 1) * 512],
                        start=(k == ks[0]), stop=(k == ks[-1]))
            for bb in range(NB):
                dst = OUTACC[:, bb * 512:(bb + 1) * 512]
                if first:
                    nc.vector.tensor_scalar_add(dst, pso[bb][:, :],
                                                BOUT[:, 0:1])
                else:
                    nc.vector.tensor_add(dst, dst, pso[bb][:, :])

        # per-window AR-independent fill, sized to each collective's
        # exposed latency (window 0 carries the one-time barrier cost)
        WINDOW_FILL = {
            0: [lambda: park_group(2),
                lambda: park_group(3),
                lambda: park_group(4),
                lambda: park_group(5)],
            1: [lambda: out_group((0, 1), True),
                lambda: park_group(6)],
            2: [lambda: park_group(7)],
            3: [lambda: out_group((2, 3))],
            7: [lambda: out_group(range(4, T_TOT - 2))],
        }

        # stage 0: everything available immediately
        psums = alloc_group()
        real_early(0, psums)
        real_late(0, psums)

        for i in range(K):
            # ---- relu + bias: PSUM -> bf16 h in SBUF ----
            # n=0 chunks on ScalarE, n=1 chunks on VectorE so both n-tiles
            # clear PSUM (and feed bn_stats) in parallel.
            hs = [hp.tile([P, BSH], BF, tag=f"h{n}", name=f"h{n}") for n in range(2)]
            sums = small.tile([P, NB], F32, tag="sums")
            sqs = small.tile([P, NB], F32, tag="sqs")
            st = small.tile([P, NB, 6], F32, tag="st")
            mv1 = small.tile([P, 2], F32, tag="mv1")
            # payload lanes: (S, B*Q, S*gamma) per n-half -- all additive
            # across cores, chosen so the post-AG coefficient chain is short
            arin = small.tile([P, 2, 3], F32, tag="arin")
            # stats payload = raw (sum, sum-of-squares) pairs, additive
            # across cores. n=0 on ScalarE (relu/square with accum_out),
            # n=1 on VectorE (relu + bn_stats), in psum-completion order so
            # both pipelines drain right behind the last matmul.
            for bb in range(NB):
                c0 = hs[0][:, bb * 512:(bb + 1) * 512]
                c1 = hs[1][:, bb * 512:(bb + 1) * 512]
                nc.scalar.activation(
                    c0, psums[0][bb][:, :], AF.Relu,
                    bias=BV[:, i, 0:1], scale=1.0,
                    accum_out=sums[:, bb:bb + 1],
                )
                scr = scrp.tile([P, 512], BF, tag="scr", name="scr")
                nc.scalar.activation(
                    scr[:, :], c0, AF.Square,
                    accum_out=sqs[:, bb:bb + 1])
                nc.vector.tensor_scalar(
                    c1, psums[1][bb][:, :], BV[:, i, 1:2], 0.0,
                    op0=OP.add, op1=OP.max,
                )
                nc.vector.bn_stats(st[:, bb, :], c1)
            nc.vector.bn_aggr(mv1[:], st[:, :, :])
            # n=1: (mean, var) -> raw (S, Q);  n=0: reduce the chunk sums
            nc.vector.tensor_scalar(
                arin[:, 1, 1:2], mv1[:, 0:1], mv1[:, 0:1], mv1[:, 1:2],
                op0=OP.mult, op1=OP.add)
            nc.vector.tensor_scalar_mul(arin[:, 1, 1:2], arin[:, 1, 1:2],
                                        float(BSH))
            nc.vector.tensor_scalar_mul(arin[:, 1, 0:1], mv1[:, 0:1],
                                        float(BSH))
            nc.vector.tensor_reduce(
                arin[:, 0, 0:1], sums[:, :], axis=mybir.AxisListType.X,
                op=OP.add)
            nc.vector.tensor_reduce(
                arin[:, 0, 1:2], sqs[:, :], axis=mybir.AxisListType.X,
                op=OP.add)
            # Q -> B*Q (both halves) and the S*gamma lane
            nc.vector.tensor_scalar_mul(arin[:, :, 1:2], arin[:, :, 1:2],
                                        float(B))
            nc.vector.tensor_mul(arin[:, :, 2], arin[:, :, 0], GV[:, i, :])

            # ---- cross-core exchange of the stats (3KB): AllGather has a
            # ~2x lower latency floor than AllReduce; the 8-way sum is done
            # locally on VectorE.  The send rides the gpsimd ring (idle);
            # the read-back is split by partition halves across the sync
            # and scalar rings to halve its latency. ----
            ccin = dram.tile([P, 2, 3], F32, tag="ccin")
            ccout = dram.tile([N_CORES, P, 2, 3], F32, tag="ccout",
                              addr_space="Shared")
            nc.sync.dma_start(ccin[:], arin[:])
            nc.gpsimd.collective_compute(
                "AllGather", OP.bypass, replica_groups=groups,
                ins=[ccin.opt()], outs=[ccout.opt()],
            )
            ag = small.tile([P, N_CORES, 2, 3], F32, tag="ag")
            HP = P // 2
            nc.sync.dma_start(ag[0:HP], ccout[:, 0:HP, :, :].rearrange(
                "r p a b -> p r a b"))

            # ---- overlap window: AR-independent matmul fill ----
            for thunk in WINDOW_FILL.get(i, []):
                thunk()
            if i < K - 1:
                psums = alloc_group()
                real_early(i + 1, psums)

            # second half of the read-back: emitted after the fill so the
            # scalar queue's park copies run during the collective
            nc.scalar.dma_start(ag[HP:P], ccout[:, HP:P, :, :].rearrange(
                "r p a b -> p r a b"))
            ared = small.tile([P, 2, 3], F32, tag="ared")
            nc.vector.tensor_reduce(
                ared[:, :, :], ag[:, :, :, :].rearrange("p r a b -> p a b r"),
                axis=mybir.AxisListType.X, op=OP.add)

            # ---- BN affine coefficients from the (S, B*Q, S*gamma)
            # global sums:  B^2*var = (B*Q) - S^2 ;
            # a = gamma/std = (B*gamma) * r ;  negc = a*mu - beta =
            # (S*gamma) * r - beta   with r = 1/sqrt(B^2*var + B^2*eps) ----
            T1 = small.tile([P, 2], F32, tag="T1")
            T3 = small.tile([P, 2], F32, tag="T3")
            rstd = small.tile([P, 2], F32, tag="rstd")
            a_ = small.tile([P, 2], F32, tag="a_")
            negc = small.tile([P, 2], F32, tag="negc")
            nc.vector.tensor_mul(T1[:], ared[:, :, 0], ared[:, :, 0])
            nc.vector.tensor_sub(T3[:], ared[:, :, 1], T1[:])
            nc.scalar.activation(rstd[:], T3[:], AF.Sqrt,
                                 bias=EPSB2[:, 0:1], scale=1.0)
            nc.vector.reciprocal(rstd[:], rstd[:])
            nc.vector.tensor_mul(a_[:], GVB[:, i, :], rstd[:])
            nc.vector.tensor_mul(negc[:], ared[:, :, 2], rstd[:])
            nc.vector.tensor_sub(negc[:], negc[:], BEV[:, i, :])

            # ---- normalize into the F blocks (bf16), chunk-major to
            # match the gated matmuls' consumption order; the two n-halves
            # run on ScalarE (out = Identity(in*scale + bias)) and VectorE
            # in parallel, halving the vector queue's normalize load ----
            nnegc = small.tile([P, 1], F32, tag="nnegc")
            nc.vector.tensor_scalar_mul(nnegc[:], negc[:, 0:1], -1.0)
            for q in range(NB):
                sl = slice(q * 512, (q + 1) * 512)
                nc.scalar.activation(
                    F[DIN // P + 2 * i][:, sl], hs[0][:, sl],
                    AF.Identity, bias=nnegc[:, 0:1], scale=a_[:, 0:1])
                nc.vector.tensor_scalar(
                    F[DIN // P + 2 * i + 1][:, sl], hs[1][:, sl],
                    a_[:, 1:2], negc[:, 1:2],
                    op0=OP.mult, op1=OP.subtract,
                )

            # ---- gated (late) matmuls of the next stage ----
            if i < K - 1:
                real_late(i + 1, psums)

        # ---- epilogue: last two output k-tiles, chunk-pipelined with the
        # final store spread across four DMA rings ----
        pso = [ps.tile([P, 512], F32, tag="pt", name="pt") for _ in range(NB)]
        oengs = [nc.sync, nc.scalar]
        for bb in range(NB):
            for k in (T_TOT - 2, T_TOT - 1):
                nc.tensor.matmul(
                    pso[bb][:, :], WO[:, k, :],
                    F[k][:, bb * 512:(bb + 1) * 512],
                    start=(k == T_TOT - 2), stop=(k == T_TOT - 1))
            dst = OUTACC[:, bb * 512:(bb + 1) * 512]
            nc.vector.tensor_add(dst, dst, pso[bb][:, :])
            # store in 256-wide half-chunks alternating rings for overlap
            for h in range(2):
                sl = slice(bb * 512 + h * 256, bb * 512 + (h + 1) * 256)
                oengs[(2 * bb + h) % 2].dma_start(outT_d[:, sl], OUTACC[:, sl])


def _get_nc():
    if "nc" not in _NC_CACHE:
        _NC_CACHE["nc"] = _build_nc()
    return _NC_CACHE["nc"]


def kernel(x, W0, W1, W2, W3, W4, W5, W6, W7, b, gamma, beta, Wout, bout):
    Ws = [W0, W1, W2, W3, W4, W5, W6, W7]
    nc = _get_nc()

    def pack_vec(v):  # [8,256] -> [128, 8, 2]
        return np.ascontiguousarray(
            np.asarray(v, np.float32).reshape(K, 2, P).transpose(2, 0, 1))

    common = {}
    for i, W in enumerate(Ws):
        wt = np.asarray(W, np.float32).T.astype(bfloat16)        # [d_i, 256]
        common[f"w{i}"] = np.ascontiguousarray(
            wt.reshape(KO[i], P, WS).transpose(1, 0, 2))         # [128, ko, 256]
    wot = np.asarray(Wout, np.float32).T.astype(bfloat16)        # [2560, 128]
    common["wo"] = np.ascontiguousarray(
        wot.reshape(T_TOT, P, DOUT).transpose(1, 0, 2))          # [128, 20, 128]
    common["bv"] = pack_vec(b)
    common["gv"] = pack_vec(gamma)
    common["bev"] = pack_vec(beta)
    common["boutv"] = np.ascontiguousarray(
        np.asarray(bout, np.float32).reshape(P, 1))

    in_maps = []
    for c in range(N_CORES):
        xs = np.asarray(x[c * BSH:(c + 1) * BSH], np.float32)    # [2048, 512]
        xt = xs.T.astype(bfloat16)                               # [512, 2048]
        in_maps.append({
            **common,
            "xt": np.ascontiguousarray(
                xt.reshape(DIN // P, P, BSH).transpose(1, 0, 2)),
        })

    kw = {}
    if TRACE_DIR is not None:
        kw = dict(trace=True, tmpdir=TRACE_DIR)
    try:
        res = run_bass_kernel_spmd(nc, in_maps, list(range(N_CORES)), **kw)
    except Exception:
        # transient PJRT INTERNAL errors have been observed; retry once
        res = run_bass_kernel_spmd(nc, in_maps, list(range(N_CORES)), **kw)
    global LAST_RESULTS
    LAST_RESULTS = res
    out = np.empty((B, DOUT), np.float32)
    for c in range(N_CORES):
        out[c * BSH:(c + 1) * BSH] = res.results[c]["outT"].T
    return out
